# revision 1
# baseline (speedup 1.0000x reference)
"""Distributed Trainium2 (Bass/Tile) kernel for a causal RoPE attention block.

Reference computation (B=2, S=2048, D=1024, H=16, HD=64):
    qkv = (x @ W_in).reshape(B,S,H,3*HD); q,k,v = split(qkv)
    q,k = rope(q,pos), rope(k,pos); q /= sqrt(HD)
    scores = q @ k^T  (causal masked); attn = softmax(scores)
    out = (attn @ v).reshape(B,S,D) @ W_out

Sharding (8 cores): core c owns batch b=c//4 and heads 4*(c%4)..4*(c%4)+3.
QKV projection is column-parallel and attention fully local per head. The
per-head context (bf16, 1MB/core) is exchanged with an AllToAll inside each
4-core batch group so every core ends up with the full context for a 512-row
sequence slice; the out-projection then runs locally against the full W_out
and the output shards concatenate on the host (no reduction outside the
device).

All matmuls run in bf16 with f32 PSUM accumulation. Softmax skips the
max-subtraction (scores are O(1) here) so exp(S) can accumulate straight
into PSUM via an appended ones-column on V that yields the row sums.
"""

import os
import sys
import numpy as np

for _p in ("/opt/trn_rl_repo", "/root/.axon_site/_ro/trn_rl_repo"):
    if _p not in sys.path and os.path.isdir(_p):
        sys.path.append(_p)

import ml_dtypes
from contextlib import ExitStack

import concourse.bass as bass
import concourse.mybir as mybir
import concourse.tile as tile
from concourse import bacc
from concourse.bass_utils import run_bass_kernel_spmd

F32 = mybir.dt.float32
BF16 = mybir.dt.bfloat16
BF = ml_dtypes.bfloat16

B, S, D, H, HD = 2, 2048, 1024, 16, 64
NCORES = 8
HPC = H // 4   # heads per core = 4
CW = HPC * HD  # per-core qkv slice width = 256
KT = 128       # k tile (partition dim of S^T tiles)
QB = 512       # q block (free dim / PSUM bank)
NKT = S // KT  # 16
NQB = S // QB  # 4
NDT = D // 128 # 8 contraction tiles
SC = S // 4    # per-core output sequence slice = 512

TRACE = False
SIM = False
# "rs": end-of-program ReduceScatter of f32 partials (proven on HW).
# "a2a": end-of-program AllToAll of bf16 context (8x less comm).
COLL = "rs"
SKEW = False    # one-stage S/AV software pipelining in attention
FASTRCP = False # custom-DVE reciprocal_approx_fast vs plain reciprocal
LAST = {}

_cache = {}


def _build(schedule, n_partial, coll):
    """schedule[(kt,qb)] in {'full','skip'} or int partial-mask index."""
    nc = bacc.Bacc(
        "TRN2", target_bir_lowering=False, debug=False, num_devices=NCORES
    )

    xT = nc.dram_tensor("xT", [D, S], BF16, kind="ExternalInput")
    wq = nc.dram_tensor("wq", [D, CW], BF16, kind="ExternalInput")
    wk = nc.dram_tensor("wk", [D, CW], BF16, kind="ExternalInput")
    wv = nc.dram_tensor("wv", [D, CW], BF16, kind="ExternalInput")
    wo_rows = D if coll == "a2a" else CW
    wo = nc.dram_tensor("wo", [wo_rows, D], BF16, kind="ExternalInput")
    tab = {}
    for t in ("cq", "sq", "ck", "sk"):
        tab[t] = nc.dram_tensor(t, [128, S], BF16, kind="ExternalInput")
    if n_partial:
        m01 = nc.dram_tensor("m01", [n_partial, KT, QB], BF16, kind="ExternalInput")
    if coll == "a2a":
        out_e = nc.dram_tensor("out", [D, SC], F32, kind="ExternalOutput")
    else:
        out_e = nc.dram_tensor("out", [D // 4, S], F32, kind="ExternalOutput")

    with tile.TileContext(nc) as tc, ExitStack() as ctx:
        cst = ctx.enter_context(tc.tile_pool(name="cst", bufs=1))
        dram = ctx.enter_context(tc.tile_pool(name="dram", bufs=1, space="DRAM"))
        qraw_p = ctx.enter_context(tc.tile_pool(name="qraw", bufs=2))
        qswp_p = ctx.enter_context(tc.tile_pool(name="qswp", bufs=2))
        rtmp_p = ctx.enter_context(tc.tile_pool(name="rtmp", bufs=3))
        e_p = ctx.enter_context(tc.tile_pool(name="e", bufs=4))
        ctmp_p = ctx.enter_context(tc.tile_pool(name="ctmp", bufs=2))
        rcp_p = ctx.enter_context(tc.tile_pool(name="rcp", bufs=2))
        rb_p = ctx.enter_context(tc.tile_pool(name="rb", bufs=2))
        oT_p = ctx.enter_context(tc.tile_pool(name="oT", bufs=3))
        mm_p = ctx.enter_context(tc.tile_pool(name="mm", bufs=4, space="PSUM"))
        cx_p = ctx.enter_context(tc.tile_pool(name="cx", bufs=2, space="PSUM"))

        # ---------------- loads (x + V weights first: V proj starts ASAP) ---
        xts, wvs = [], []
        for d in range(NDT):
            t = cst.tile([128, S], BF16, tag=f"xT{d}", name=f"xT{d}")
            nc.sync.dma_start(t[:], xT.ap()[d * 128:(d + 1) * 128, :])
            xts.append(t)
            t = cst.tile([128, CW], BF16, tag=f"wv{d}", name=f"wv{d}")
            nc.sync.dma_start(t[:], wv.ap()[d * 128:(d + 1) * 128, :])
            wvs.append(t)
        wqs, wks = [], []
        for nm, dram_t, lst in (("wq", wq, wqs), ("wk", wk, wks)):
            for d in range(NDT):
                t = cst.tile([128, CW], BF16, tag=f"{nm}{d}", name=f"{nm}{d}")
                nc.sync.dma_start(t[:], dram_t.ap()[d * 128:(d + 1) * 128, :])
                lst.append(t)
        tabs = {}
        for tn in ("cq", "sq", "ck", "sk"):
            t = cst.tile([128, S], BF16, tag=tn, name=f"tab_{tn}")
            nc.sync.dma_start(t[:], tab[tn].ap()[:, :])
            tabs[tn] = t
        mts = []
        for i in range(n_partial):
            t = cst.tile([KT, QB], BF16, tag=f"m{i}", name=f"m{i}")
            nc.sync.dma_start(t[:], m01.ap()[i])
            mts.append(t)

        # ---------------- V projection (natural layout + ones column) ------
        vplus = []
        for st in range(NKT):
            vp = cst.tile([128, HPC * 65], BF16, tag=f"vp{st}", name=f"vp{st}")
            nc.vector.memset(vp[:], 1.0)
            vps = mm_p.tile([128, CW], F32, tag="mm", name=f"vps{st}")
            for d in range(NDT):
                nc.tensor.matmul(
                    vps[:], xts[d][:, st * 128:(st + 1) * 128], wvs[d][:],
                    start=(d == 0), stop=(d == NDT - 1),
                )
            for hl in range(HPC):
                nc.vector.tensor_copy(
                    vp[:, 65 * hl:65 * hl + 64], vps[:, 64 * hl:64 * hl + 64]
                )
            vplus.append(vp)

        # ---------------- Q/K projection + RoPE (bf16) ----------------
        qrot, krot = [], []
        for i in range(2):
            qrot.append(cst.tile([128, S], BF16, tag=f"qr{i}", name=f"qr{i}"))
            krot.append(cst.tile([128, S], BF16, tag=f"kr{i}", name=f"kr{i}"))

        for which, ws, ctab, stab, rots in (
            ("q", wqs, tabs["cq"], tabs["sq"], qrot),
            ("k", wks, tabs["ck"], tabs["sk"], krot),
        ):
            for et in range(2):
                raw = qraw_p.tile([128, S], BF16, tag="qraw", name=f"raw_{which}{et}")
                for sb in range(NQB):
                    ps = mm_p.tile([128, QB], F32, tag="mm", name=f"pj_{which}{et}{sb}")
                    for d in range(NDT):
                        nc.tensor.matmul(
                            ps[:], ws[d][:, et * 128:(et + 1) * 128],
                            xts[d][:, sb * QB:(sb + 1) * QB],
                            start=(d == 0), stop=(d == NDT - 1),
                        )
                    nc.vector.tensor_copy(raw[:, sb * QB:(sb + 1) * QB], ps[:])
                # rotate-half partner: swap 32-row halves within each 64-row head
                swp = qswp_p.tile([128, S], BF16, tag="qswp", name=f"swp_{which}{et}")
                for g in range(4):
                    src = (g ^ 1) * 32
                    nc.sync.dma_start(
                        swp[g * 32:(g + 1) * 32, :], raw[src:src + 32, :]
                    )
                # rot = raw*C + swp*Ssig   (C/Ssig fold the q scaling by 1/8)
                t1 = rtmp_p.tile([128, S], BF16, tag="rtmp", name=f"t1{which}{et}")
                t2 = rtmp_p.tile([128, S], BF16, tag="rtmp", name=f"t2{which}{et}")
                nc.vector.tensor_mul(t1[:], raw[:], ctab[:])
                nc.gpsimd.tensor_mul(t2[:], swp[:], stab[:])
                nc.vector.tensor_add(rots[et][:], t1[:], t2[:])

        # ---------------- attention ----------------
        # ctx2[i]: [128, S] bf16 — context^T for heads (2i, 2i+1).
        ctx2 = [
            cst.tile([128, S], BF16, tag=f"cx{i}", name=f"ctx2_{i}")
            for i in range(2)
        ]
        for hl in range(HPC):
            i, r0 = hl // 2, (hl % 2) * 64
            psl = slice(r0, r0 + 64)
            for qb in range(NQB):
                qsl = slice(qb * QB, (qb + 1) * QB)
                kts = [kt for kt in range(NKT) if schedule[(kt, qb)] != "skip"]
                cps = cx_p.tile([65, QB], F32, tag="cx", name=f"cps{qb}{hl}")
                # one-stage S/AV skew keeps PE busy while ACT runs exp
                pend = None
                for n, kt in enumerate(kts):
                    sps = mm_p.tile([KT, QB], F32, tag="mm", name=f"sps{qb}{hl}{kt}")
                    nc.tensor.matmul(
                        sps[:], krot[i][psl, kt * KT:(kt + 1) * KT],
                        qrot[i][psl, qsl], start=True, stop=True,
                    )
                    e = e_p.tile([KT, QB], BF16, tag="e", name=f"e{qb}{hl}{kt}")
                    nc.scalar.activation(
                        e[:], sps[:], mybir.ActivationFunctionType.Exp
                    )
                    cls = schedule[(kt, qb)]
                    if cls != "full":
                        nc.vector.tensor_mul(e[:], e[:], mts[cls][:])
                    av = (vplus[kt][:, 65 * hl:65 * hl + 65], e)
                    if SKEW:
                        if pend is not None:
                            nc.tensor.matmul(
                                cps[:], pend[0], pend[1],
                                start=(n == 1), stop=False,
                            )
                        pend = av
                    else:
                        nc.tensor.matmul(
                            cps[:], av[0], av[1],
                            start=(n == 0), stop=(n == len(kts) - 1),
                        )
                if SKEW:
                    nc.tensor.matmul(
                        cps[:], pend[0], pend[1],
                        start=(len(kts) == 1), stop=True,
                    )
                # normalize: ctx[d,q] / sigma[q] (sigma = row 64 of cps)
                rcp = rcp_p.tile([1, QB], F32, tag="rcp", name=f"rcp{qb}{hl}")
                if FASTRCP:
                    nc.vector.reciprocal_approx_fast(rcp[:], cps[64:65, :])
                else:
                    nc.vector.reciprocal(rcp[:], cps[64:65, :])
                rb = rb_p.tile([64, QB], F32, tag="rb", name=f"rb{qb}{hl}")
                nc.gpsimd.partition_broadcast(rb[:], rcp[:])
                if r0 == 0:
                    nc.vector.tensor_mul(ctx2[i][0:64, qsl], cps[0:64, :], rb[:])
                else:
                    ct = ctmp_p.tile([64, QB], BF16, tag="ctmp", name=f"ct{qb}{hl}")
                    nc.vector.tensor_mul(ct[:], cps[0:64, :], rb[:])
                    nc.sync.dma_start(ctx2[i][64:128, qsl], ct[:])
        if coll == "a2a":
            # 8-core AllToAll at end of attention: every core outputs 256 seq
            # rows of BOTH batches, so all 8 chunks carry useful context
            # (group g cores hold batch g's ctx). One collective, both
            # head-pairs stacked: chunk p = [pair0 rows; pair1 rows].
            SH = S // NCORES  # 256
            a2a_in = dram.tile([NCORES, 256, SH], BF16, tag="a2i", name="a2a_in")
            a2a_out = dram.tile([NCORES, 256, SH], BF16, tag="a2o", name="a2a_out")
            for p in range(NCORES):
                for i in range(2):
                    nc.sync.dma_start(
                        a2a_in[p, 128 * i:128 * (i + 1), :],
                        ctx2[i][:, p * SH:(p + 1) * SH],
                    )
            nc.gpsimd.collective_compute(
                "AllToAll",
                mybir.AluOpType.bypass,
                replica_groups=[list(range(NCORES))],
                ins=[a2a_in.opt()],
                outs=[a2a_out.opt()],
            )
            # out-projection on the gathered context
            wos = []
            for j in range(NDT):
                t = cst.tile([128, D], BF16, tag=f"wo{j}", name=f"wo{j}")
                nc.sync.dma_start(t[:], wo.ap()[j * 128:(j + 1) * 128, :])
                wos.append(t)
            # cxf[b][j]: c-tile j of batch b's full context for this core's
            # seq slice; from peer p = 4b + j//2, head-pair j%2.
            cxf = [[None] * NDT for _ in range(2)]
            for b in range(2):
                for j in range(NDT):
                    t = cst.tile(
                        [128, SH], BF16, tag=f"cxf{b}_{j}", name=f"cxf{b}_{j}"
                    )
                    i = j % 2
                    nc.sync.dma_start(
                        t[:], a2a_out[4 * b + j // 2, 128 * i:128 * (i + 1), :]
                    )
                    cxf[b][j] = t
            for ot in range(NDT):
                osl = slice(ot * 128, (ot + 1) * 128)
                for b in range(2):
                    ops = mm_p.tile([128, SH], F32, tag="mm", name=f"ops{ot}_{b}")
                    for j in range(NDT):
                        nc.tensor.matmul(
                            ops[:], wos[j][:, osl], cxf[b][j][:],
                            start=(j == 0), stop=(j == NDT - 1),
                        )
                    oT = oT_p.tile([128, SH], F32, tag="oT", name=f"oT{ot}_{b}")
                    nc.vector.tensor_copy(oT[:], ops[:])
                    nc.sync.dma_start(
                        out_e.ap()[osl, b * SH:(b + 1) * SH], oT[:]
                    )
        else:
            # row-parallel out-projection + f32 ReduceScatter in batch groups
            wos = []
            for i in range(2):
                t = cst.tile([128, D], BF16, tag=f"wo{i}", name=f"wo{i}")
                nc.sync.dma_start(t[:], wo.ap()[i * 128:(i + 1) * 128, :])
                wos.append(t)
            rs_in = dram.tile([D, S], F32, tag="rs_in", name="rs_in")
            for qb in range(NQB):
                qsl = slice(qb * QB, (qb + 1) * QB)
                for ot in range(NDT):
                    osl = slice(ot * 128, (ot + 1) * 128)
                    ops = mm_p.tile([128, QB], F32, tag="mm", name=f"ops{qb}{ot}")
                    for i in range(2):
                        nc.tensor.matmul(
                            ops[:], wos[i][:, osl], ctx2[i][:, qsl],
                            start=(i == 0), stop=(i == 1),
                        )
                    oT = oT_p.tile([128, QB], F32, tag="oT", name=f"oT{qb}{ot}")
                    nc.vector.tensor_copy(oT[:], ops[:])
                    nc.sync.dma_start(rs_in[osl, qsl], oT[:])
            rs_out = dram.tile([D // 4, S], F32, tag="rs_out", name="rs_out")
            nc.gpsimd.collective_compute(
                "ReduceScatter",
                mybir.AluOpType.add,
                replica_groups=[[0, 1, 2, 3], [4, 5, 6, 7]],
                ins=[rs_in.opt()],
                outs=[rs_out.opt()],
            )
            nc.sync.dma_start(out_e.ap()[:, :], rs_out[:])

    nc.compile()
    return nc


def _classify_mask(mask):
    """Per (kt,qb) tile classification + packed partial tiles (S^T layout)."""
    m2 = np.asarray(mask).reshape(S, S)  # [q, k] bool
    schedule = {}
    partials = []
    for kt in range(NKT):
        for qb in range(NQB):
            sub = m2[qb * QB:(qb + 1) * QB, kt * KT:(kt + 1) * KT]
            if sub.all():
                schedule[(kt, qb)] = "full"
            elif not sub.any():
                schedule[(kt, qb)] = "skip"
            else:
                schedule[(kt, qb)] = len(partials)
                partials.append(np.ascontiguousarray(sub.T).astype(BF))
    m01 = (
        np.stack(partials)
        if partials
        else np.zeros((0, KT, QB), dtype=BF)
    )
    return schedule, m01


def kernel(inputs, segment_positions, mask, W_in, W_out):
    inputs = np.asarray(inputs, dtype=np.float32)
    segment_positions = np.asarray(segment_positions, dtype=np.int32)
    W_in = np.asarray(W_in, dtype=np.float32)
    W_out = np.asarray(W_out, dtype=np.float32)

    schedule, m01 = _classify_mask(mask)
    key = (COLL, tuple(sorted(schedule.items())))
    if key not in _cache:
        _cache[key] = _build(schedule, m01.shape[0], COLL)
    nc = _cache[key]

    # ---- host-side shard prep (layout/dtype only; no math beyond tables) ----
    # W_in column e maps to head e//192, role (e%192)//64 (q/k/v), dim e%64.
    Wr = W_in.reshape(D, H, 3, HD)
    half = HD // 2
    inv_freq = (1.0 / (10000.0 ** (np.arange(half, dtype=np.float32) / half)))
    wo_full = np.ascontiguousarray(W_out).astype(BF)

    in_maps = []
    for c in range(NCORES):
        b, h0 = c // 4, HPC * (c % 4)
        woc = (
            wo_full
            if COLL == "a2a"
            else np.ascontiguousarray(W_out[h0 * HD:(h0 + HPC) * HD, :]).astype(BF)
        )
        xTc = np.ascontiguousarray(inputs[b].T).astype(BF)
        wqc = np.ascontiguousarray(Wr[:, h0:h0 + HPC, 0, :].reshape(D, CW)).astype(BF)
        wkc = np.ascontiguousarray(Wr[:, h0:h0 + HPC, 1, :].reshape(D, CW)).astype(BF)
        wvc = np.ascontiguousarray(Wr[:, h0:h0 + HPC, 2, :].reshape(D, CW)).astype(BF)

        ang = segment_positions[b].astype(np.float32)[None, :] * inv_freq[:, None]
        c_, s_ = np.cos(ang), np.sin(ang)  # [32, S]
        C64 = np.vstack([c_, c_])
        S64 = np.vstack([-s_, s_])
        C128 = np.vstack([C64, C64]).astype(np.float32)
        S128 = np.vstack([S64, S64]).astype(np.float32)
        scale = 1.0 / np.sqrt(HD).astype(np.float32)
        im = {
            "xT": xTc, "wq": wqc, "wk": wkc, "wv": wvc, "wo": woc,
            "cq": (C128 * scale).astype(BF), "sq": (S128 * scale).astype(BF),
            "ck": C128.astype(BF), "sk": S128.astype(BF),
        }
        if m01.shape[0]:
            im["m01"] = m01
        in_maps.append(im)

    if SIM:
        from concourse import bass_interp

        sim = bass_interp.MultiCoreSim(nc, NCORES)
        for c in range(NCORES):
            for k, v in in_maps[c].items():
                sim.cores[c].tensor(k)[:] = v
        sim.simulate(check_with_hw=False)
        results = [
            {"out": np.asarray(sim.cores[c].mem_tensor("out"))}
            for c in range(NCORES)
        ]
        LAST["exec_time_ns"] = None
    else:
        res = run_bass_kernel_spmd(
            nc, in_maps, core_ids=list(range(NCORES)), trace=TRACE
        )
        LAST["exec_time_ns"] = res.exec_time_ns
        LAST["results"] = res
        results = res.results

    out = np.empty((B, S, D), dtype=np.float32)
    if COLL == "a2a":
        # core c returns out^T [D, 512]: cols 0-255 = batch 0 rows 256c..,
        # cols 256-511 = batch 1 rows 256c..
        SH = S // NCORES
        for c in range(NCORES):
            r = np.asarray(results[c]["out"])
            for b in range(B):
                out[b, c * SH:(c + 1) * SH, :] = r[:, b * SH:(b + 1) * SH].T
    else:
        # group g covers batch g; rank r in group returns out^T rows 256r..
        for b in range(B):
            outT = np.concatenate(
                [np.asarray(results[4 * b + i]["out"]) for i in range(4)], axis=0
            )
            out[b] = outT.T
    return out



# revision 2
# speedup vs baseline: 1.3887x; 1.3887x over previous
"""Distributed Trainium2 (Bass/Tile) kernel for a causal RoPE attention block.

Reference computation (B=2, S=2048, D=1024, H=16, HD=64):
    qkv = (x @ W_in).reshape(B,S,H,3*HD); q,k,v = split(qkv)
    q,k = rope(q,pos), rope(k,pos); q /= sqrt(HD)
    scores = q @ k^T  (causal masked); attn = softmax(scores)
    out = (attn @ v).reshape(B,S,D) @ W_out

Sharding (8 cores): core c owns batch b=c//4 and heads 4*(c%4)..4*(c%4)+3.
QKV projection is column-parallel and attention fully local per head. The
per-head context (bf16, 1MB/core) is exchanged with an AllToAll inside each
4-core batch group so every core ends up with the full context for a 512-row
sequence slice; the out-projection then runs locally against the full W_out
and the output shards concatenate on the host (no reduction outside the
device).

All matmuls run in bf16 with f32 PSUM accumulation. Softmax skips the
max-subtraction (scores are O(1) here) so exp(S) can accumulate straight
into PSUM via an appended ones-column on V that yields the row sums.
"""

import os
import sys
import numpy as np

for _p in ("/opt/trn_rl_repo", "/root/.axon_site/_ro/trn_rl_repo"):
    if _p not in sys.path and os.path.isdir(_p):
        sys.path.append(_p)

import ml_dtypes
from contextlib import ExitStack

import concourse.bass as bass
import concourse.mybir as mybir
import concourse.tile as tile
from concourse import bacc
from concourse.bass_utils import run_bass_kernel_spmd

F32 = mybir.dt.float32
BF16 = mybir.dt.bfloat16
BF = ml_dtypes.bfloat16

B, S, D, H, HD = 2, 2048, 1024, 16, 64
NCORES = 8
HPC = H // 4   # heads per core = 4
CW = HPC * HD  # per-core qkv slice width = 256
KT = 128       # k tile (partition dim of S^T tiles)
QB = 512       # q block (free dim / PSUM bank)
NKT = S // KT  # 16
NQB = S // QB  # 4
NDT = D // 128 # 8 contraction tiles
SC = S // 4    # per-core output sequence slice = 512

TRACE = False
SIM = False
# "rs": end-of-program ReduceScatter of f32 partials (proven on HW).
# "a2a": end-of-program AllToAll of bf16 context (8x less comm).
COLL = "a2a"
SKEW = False    # one-stage S/AV software pipelining in attention
FASTRCP = False # custom-DVE reciprocal_approx_fast vs plain reciprocal
LAST = {}

_cache = {}


def _build(schedule, n_partial, coll):
    """schedule[(kt,qb)] in {'full','skip'} or int partial-mask index."""
    nc = bacc.Bacc(
        "TRN2", target_bir_lowering=False, debug=False, num_devices=NCORES
    )

    xT = nc.dram_tensor("xT", [D, S], BF16, kind="ExternalInput")
    wq = nc.dram_tensor("wq", [D, CW], BF16, kind="ExternalInput")
    wk = nc.dram_tensor("wk", [D, CW], BF16, kind="ExternalInput")
    wv = nc.dram_tensor("wv", [D, CW], BF16, kind="ExternalInput")
    wo_rows = D if coll == "a2a" else CW
    wo = nc.dram_tensor("wo", [wo_rows, D], BF16, kind="ExternalInput")
    tab = {}
    for t in ("cq", "sq", "ck", "sk"):
        tab[t] = nc.dram_tensor(t, [128, S], BF16, kind="ExternalInput")
    if n_partial:
        m01 = nc.dram_tensor("m01", [n_partial, KT, QB], BF16, kind="ExternalInput")
    if coll == "a2a":
        out_e = nc.dram_tensor("out", [D, SC], F32, kind="ExternalOutput")
    else:
        out_e = nc.dram_tensor("out", [D // 4, S], F32, kind="ExternalOutput")

    with tile.TileContext(nc) as tc, ExitStack() as ctx:
        cst = ctx.enter_context(tc.tile_pool(name="cst", bufs=1))
        dram = ctx.enter_context(tc.tile_pool(name="dram", bufs=1, space="DRAM"))
        qraw_p = ctx.enter_context(tc.tile_pool(name="qraw", bufs=2))
        qswp_p = ctx.enter_context(tc.tile_pool(name="qswp", bufs=2))
        rtmp_p = ctx.enter_context(tc.tile_pool(name="rtmp", bufs=3))
        e_p = ctx.enter_context(tc.tile_pool(name="e", bufs=4))
        ctmp_p = ctx.enter_context(tc.tile_pool(name="ctmp", bufs=2))
        rcp_p = ctx.enter_context(tc.tile_pool(name="rcp", bufs=2))
        rb_p = ctx.enter_context(tc.tile_pool(name="rb", bufs=2))
        oT_p = ctx.enter_context(tc.tile_pool(name="oT", bufs=3))
        mm_p = ctx.enter_context(tc.tile_pool(name="mm", bufs=4, space="PSUM"))
        cx_p = ctx.enter_context(tc.tile_pool(name="cx", bufs=2, space="PSUM"))

        # ---------------- loads (x + V weights first: V proj starts ASAP) ---
        xts, wvs = [], []
        for d in range(NDT):
            t = cst.tile([128, S], BF16, tag=f"xT{d}", name=f"xT{d}")
            nc.sync.dma_start(t[:], xT.ap()[d * 128:(d + 1) * 128, :])
            xts.append(t)
            t = cst.tile([128, CW], BF16, tag=f"wv{d}", name=f"wv{d}")
            nc.sync.dma_start(t[:], wv.ap()[d * 128:(d + 1) * 128, :])
            wvs.append(t)
        wqs, wks = [], []
        for nm, dram_t, lst in (("wq", wq, wqs), ("wk", wk, wks)):
            for d in range(NDT):
                t = cst.tile([128, CW], BF16, tag=f"{nm}{d}", name=f"{nm}{d}")
                nc.sync.dma_start(t[:], dram_t.ap()[d * 128:(d + 1) * 128, :])
                lst.append(t)
        tabs = {}
        for tn in ("cq", "sq", "ck", "sk"):
            t = cst.tile([128, S], BF16, tag=tn, name=f"tab_{tn}")
            nc.sync.dma_start(t[:], tab[tn].ap()[:, :])
            tabs[tn] = t
        mts = []
        for i in range(n_partial):
            t = cst.tile([KT, QB], BF16, tag=f"m{i}", name=f"m{i}")
            nc.sync.dma_start(t[:], m01.ap()[i])
            mts.append(t)

        # ---------------- V projection (natural layout + ones column) ------
        vplus = []
        for st in range(NKT):
            vp = cst.tile([128, HPC * 65], BF16, tag=f"vp{st}", name=f"vp{st}")
            nc.vector.memset(vp[:], 1.0)
            vps = mm_p.tile([128, CW], F32, tag="mm", name=f"vps{st}")
            for d in range(NDT):
                nc.tensor.matmul(
                    vps[:], xts[d][:, st * 128:(st + 1) * 128], wvs[d][:],
                    start=(d == 0), stop=(d == NDT - 1),
                )
            for hl in range(HPC):
                nc.vector.tensor_copy(
                    vp[:, 65 * hl:65 * hl + 64], vps[:, 64 * hl:64 * hl + 64]
                )
            vplus.append(vp)

        # ---------------- Q/K projection + RoPE (bf16) ----------------
        qrot, krot = [], []
        for i in range(2):
            qrot.append(cst.tile([128, S], BF16, tag=f"qr{i}", name=f"qr{i}"))
            krot.append(cst.tile([128, S], BF16, tag=f"kr{i}", name=f"kr{i}"))

        for which, ws, ctab, stab, rots in (
            ("q", wqs, tabs["cq"], tabs["sq"], qrot),
            ("k", wks, tabs["ck"], tabs["sk"], krot),
        ):
            for et in range(2):
                raw = qraw_p.tile([128, S], BF16, tag="qraw", name=f"raw_{which}{et}")
                for sb in range(NQB):
                    ps = mm_p.tile([128, QB], F32, tag="mm", name=f"pj_{which}{et}{sb}")
                    for d in range(NDT):
                        nc.tensor.matmul(
                            ps[:], ws[d][:, et * 128:(et + 1) * 128],
                            xts[d][:, sb * QB:(sb + 1) * QB],
                            start=(d == 0), stop=(d == NDT - 1),
                        )
                    nc.vector.tensor_copy(raw[:, sb * QB:(sb + 1) * QB], ps[:])
                # rotate-half partner: swap 32-row halves within each 64-row head
                swp = qswp_p.tile([128, S], BF16, tag="qswp", name=f"swp_{which}{et}")
                for g in range(4):
                    src = (g ^ 1) * 32
                    nc.sync.dma_start(
                        swp[g * 32:(g + 1) * 32, :], raw[src:src + 32, :]
                    )
                # rot = raw*C + swp*Ssig   (C/Ssig fold the q scaling by 1/8)
                t1 = rtmp_p.tile([128, S], BF16, tag="rtmp", name=f"t1{which}{et}")
                t2 = rtmp_p.tile([128, S], BF16, tag="rtmp", name=f"t2{which}{et}")
                nc.vector.tensor_mul(t1[:], raw[:], ctab[:])
                nc.gpsimd.tensor_mul(t2[:], swp[:], stab[:])
                nc.vector.tensor_add(rots[et][:], t1[:], t2[:])

        # ---------------- attention ----------------
        # ctx2[i]: [128, S] bf16 — context^T for heads (2i, 2i+1).
        ctx2 = [
            cst.tile([128, S], BF16, tag=f"cx{i}", name=f"ctx2_{i}")
            for i in range(2)
        ]
        for hl in range(HPC):
            i, r0 = hl // 2, (hl % 2) * 64
            psl = slice(r0, r0 + 64)
            for qb in range(NQB):
                qsl = slice(qb * QB, (qb + 1) * QB)
                kts = [kt for kt in range(NKT) if schedule[(kt, qb)] != "skip"]
                cps = cx_p.tile([65, QB], F32, tag="cx", name=f"cps{qb}{hl}")
                # one-stage S/AV skew keeps PE busy while ACT runs exp
                pend = None
                for n, kt in enumerate(kts):
                    sps = mm_p.tile([KT, QB], F32, tag="mm", name=f"sps{qb}{hl}{kt}")
                    nc.tensor.matmul(
                        sps[:], krot[i][psl, kt * KT:(kt + 1) * KT],
                        qrot[i][psl, qsl], start=True, stop=True,
                    )
                    e = e_p.tile([KT, QB], BF16, tag="e", name=f"e{qb}{hl}{kt}")
                    nc.scalar.activation(
                        e[:], sps[:], mybir.ActivationFunctionType.Exp
                    )
                    cls = schedule[(kt, qb)]
                    if cls != "full":
                        nc.vector.tensor_mul(e[:], e[:], mts[cls][:])
                    av = (vplus[kt][:, 65 * hl:65 * hl + 65], e)
                    if SKEW:
                        if pend is not None:
                            nc.tensor.matmul(
                                cps[:], pend[0], pend[1],
                                start=(n == 1), stop=False,
                            )
                        pend = av
                    else:
                        nc.tensor.matmul(
                            cps[:], av[0], av[1],
                            start=(n == 0), stop=(n == len(kts) - 1),
                        )
                if SKEW:
                    nc.tensor.matmul(
                        cps[:], pend[0], pend[1],
                        start=(len(kts) == 1), stop=True,
                    )
                # normalize: ctx[d,q] / sigma[q] (sigma = row 64 of cps)
                rcp = rcp_p.tile([1, QB], F32, tag="rcp", name=f"rcp{qb}{hl}")
                if FASTRCP:
                    nc.vector.reciprocal_approx_fast(rcp[:], cps[64:65, :])
                else:
                    nc.vector.reciprocal(rcp[:], cps[64:65, :])
                rb = rb_p.tile([64, QB], F32, tag="rb", name=f"rb{qb}{hl}")
                nc.gpsimd.partition_broadcast(rb[:], rcp[:])
                if r0 == 0:
                    nc.vector.tensor_mul(ctx2[i][0:64, qsl], cps[0:64, :], rb[:])
                else:
                    ct = ctmp_p.tile([64, QB], BF16, tag="ctmp", name=f"ct{qb}{hl}")
                    nc.vector.tensor_mul(ct[:], cps[0:64, :], rb[:])
                    nc.sync.dma_start(ctx2[i][64:128, qsl], ct[:])
        if coll == "a2a":
            # 8-core AllToAll at end of attention: every core outputs 256 seq
            # rows of BOTH batches, so all 8 chunks carry useful context
            # (group g cores hold batch g's ctx). One collective, both
            # head-pairs stacked: chunk p = [pair0 rows; pair1 rows].
            SH = S // NCORES  # 256
            a2a_in = dram.tile([NCORES, 256, SH], BF16, tag="a2i", name="a2a_in")
            a2a_out = dram.tile([NCORES, 256, SH], BF16, tag="a2o", name="a2a_out")
            for p in range(NCORES):
                for i in range(2):
                    nc.sync.dma_start(
                        a2a_in[p, 128 * i:128 * (i + 1), :],
                        ctx2[i][:, p * SH:(p + 1) * SH],
                    )
            nc.gpsimd.collective_compute(
                "AllToAll",
                mybir.AluOpType.bypass,
                replica_groups=[list(range(NCORES))],
                ins=[a2a_in.opt()],
                outs=[a2a_out.opt()],
            )
            # out-projection on the gathered context
            wos = []
            for j in range(NDT):
                t = cst.tile([128, D], BF16, tag=f"wo{j}", name=f"wo{j}")
                nc.sync.dma_start(t[:], wo.ap()[j * 128:(j + 1) * 128, :])
                wos.append(t)
            # cxf[b][j]: c-tile j of batch b's full context for this core's
            # seq slice; from peer p = 4b + j//2, head-pair j%2.
            cxf = [[None] * NDT for _ in range(2)]
            for b in range(2):
                for j in range(NDT):
                    t = cst.tile(
                        [128, SH], BF16, tag=f"cxf{b}_{j}", name=f"cxf{b}_{j}"
                    )
                    i = j % 2
                    nc.sync.dma_start(
                        t[:], a2a_out[4 * b + j // 2, 128 * i:128 * (i + 1), :]
                    )
                    cxf[b][j] = t
            for ot in range(NDT):
                osl = slice(ot * 128, (ot + 1) * 128)
                for b in range(2):
                    ops = mm_p.tile([128, SH], F32, tag="mm", name=f"ops{ot}_{b}")
                    for j in range(NDT):
                        nc.tensor.matmul(
                            ops[:], wos[j][:, osl], cxf[b][j][:],
                            start=(j == 0), stop=(j == NDT - 1),
                        )
                    oT = oT_p.tile([128, SH], F32, tag="oT", name=f"oT{ot}_{b}")
                    nc.vector.tensor_copy(oT[:], ops[:])
                    nc.sync.dma_start(
                        out_e.ap()[osl, b * SH:(b + 1) * SH], oT[:]
                    )
        else:
            # row-parallel out-projection + f32 ReduceScatter in batch groups
            wos = []
            for i in range(2):
                t = cst.tile([128, D], BF16, tag=f"wo{i}", name=f"wo{i}")
                nc.sync.dma_start(t[:], wo.ap()[i * 128:(i + 1) * 128, :])
                wos.append(t)
            rs_in = dram.tile([D, S], F32, tag="rs_in", name="rs_in")
            for qb in range(NQB):
                qsl = slice(qb * QB, (qb + 1) * QB)
                for ot in range(NDT):
                    osl = slice(ot * 128, (ot + 1) * 128)
                    ops = mm_p.tile([128, QB], F32, tag="mm", name=f"ops{qb}{ot}")
                    for i in range(2):
                        nc.tensor.matmul(
                            ops[:], wos[i][:, osl], ctx2[i][:, qsl],
                            start=(i == 0), stop=(i == 1),
                        )
                    oT = oT_p.tile([128, QB], F32, tag="oT", name=f"oT{qb}{ot}")
                    nc.vector.tensor_copy(oT[:], ops[:])
                    nc.sync.dma_start(rs_in[osl, qsl], oT[:])
            rs_out = dram.tile([D // 4, S], F32, tag="rs_out", name="rs_out")
            nc.gpsimd.collective_compute(
                "ReduceScatter",
                mybir.AluOpType.add,
                replica_groups=[[0, 1, 2, 3], [4, 5, 6, 7]],
                ins=[rs_in.opt()],
                outs=[rs_out.opt()],
            )
            nc.sync.dma_start(out_e.ap()[:, :], rs_out[:])

    nc.compile()
    return nc


def _classify_mask(mask):
    """Per (kt,qb) tile classification + packed partial tiles (S^T layout)."""
    m2 = np.asarray(mask).reshape(S, S)  # [q, k] bool
    schedule = {}
    partials = []
    for kt in range(NKT):
        for qb in range(NQB):
            sub = m2[qb * QB:(qb + 1) * QB, kt * KT:(kt + 1) * KT]
            if sub.all():
                schedule[(kt, qb)] = "full"
            elif not sub.any():
                schedule[(kt, qb)] = "skip"
            else:
                schedule[(kt, qb)] = len(partials)
                partials.append(np.ascontiguousarray(sub.T).astype(BF))
    m01 = (
        np.stack(partials)
        if partials
        else np.zeros((0, KT, QB), dtype=BF)
    )
    return schedule, m01


def kernel(inputs, segment_positions, mask, W_in, W_out):
    inputs = np.asarray(inputs, dtype=np.float32)
    segment_positions = np.asarray(segment_positions, dtype=np.int32)
    W_in = np.asarray(W_in, dtype=np.float32)
    W_out = np.asarray(W_out, dtype=np.float32)

    schedule, m01 = _classify_mask(mask)
    key = (COLL, tuple(sorted(schedule.items())))
    if key not in _cache:
        _cache[key] = _build(schedule, m01.shape[0], COLL)
    nc = _cache[key]

    # ---- host-side shard prep (layout/dtype only; no math beyond tables) ----
    # W_in column e maps to head e//192, role (e%192)//64 (q/k/v), dim e%64.
    Wr = W_in.reshape(D, H, 3, HD)
    half = HD // 2
    inv_freq = (1.0 / (10000.0 ** (np.arange(half, dtype=np.float32) / half)))
    wo_full = np.ascontiguousarray(W_out).astype(BF)

    in_maps = []
    for c in range(NCORES):
        b, h0 = c // 4, HPC * (c % 4)
        woc = (
            wo_full
            if COLL == "a2a"
            else np.ascontiguousarray(W_out[h0 * HD:(h0 + HPC) * HD, :]).astype(BF)
        )
        xTc = np.ascontiguousarray(inputs[b].T).astype(BF)
        wqc = np.ascontiguousarray(Wr[:, h0:h0 + HPC, 0, :].reshape(D, CW)).astype(BF)
        wkc = np.ascontiguousarray(Wr[:, h0:h0 + HPC, 1, :].reshape(D, CW)).astype(BF)
        wvc = np.ascontiguousarray(Wr[:, h0:h0 + HPC, 2, :].reshape(D, CW)).astype(BF)

        ang = segment_positions[b].astype(np.float32)[None, :] * inv_freq[:, None]
        c_, s_ = np.cos(ang), np.sin(ang)  # [32, S]
        C64 = np.vstack([c_, c_])
        S64 = np.vstack([-s_, s_])
        C128 = np.vstack([C64, C64]).astype(np.float32)
        S128 = np.vstack([S64, S64]).astype(np.float32)
        scale = 1.0 / np.sqrt(HD).astype(np.float32)
        im = {
            "xT": xTc, "wq": wqc, "wk": wkc, "wv": wvc, "wo": woc,
            "cq": (C128 * scale).astype(BF), "sq": (S128 * scale).astype(BF),
            "ck": C128.astype(BF), "sk": S128.astype(BF),
        }
        if m01.shape[0]:
            im["m01"] = m01
        in_maps.append(im)

    if SIM:
        from concourse import bass_interp

        sim = bass_interp.MultiCoreSim(nc, NCORES)
        for c in range(NCORES):
            for k, v in in_maps[c].items():
                sim.cores[c].tensor(k)[:] = v
        sim.simulate(check_with_hw=False)
        results = [
            {"out": np.asarray(sim.cores[c].mem_tensor("out"))}
            for c in range(NCORES)
        ]
        LAST["exec_time_ns"] = None
    else:
        res = run_bass_kernel_spmd(
            nc, in_maps, core_ids=list(range(NCORES)), trace=TRACE
        )
        LAST["exec_time_ns"] = res.exec_time_ns
        LAST["results"] = res
        results = res.results

    out = np.empty((B, S, D), dtype=np.float32)
    if COLL == "a2a":
        # core c returns out^T [D, 512]: cols 0-255 = batch 0 rows 256c..,
        # cols 256-511 = batch 1 rows 256c..
        SH = S // NCORES
        for c in range(NCORES):
            r = np.asarray(results[c]["out"])
            for b in range(B):
                out[b, c * SH:(c + 1) * SH, :] = r[:, b * SH:(b + 1) * SH].T
    else:
        # group g covers batch g; rank r in group returns out^T rows 256r..
        for b in range(B):
            outT = np.concatenate(
                [np.asarray(results[4 * b + i]["out"]) for i in range(4)], axis=0
            )
            out[b] = outT.T
    return out

